# revision 22
# baseline (speedup 1.0000x reference)
# v5.2: single combined K+V AllGather in bf16 over 4-rank replica groups.
# Data-parallel over tokens (8 cores x 512 tokens, 4 cores per batch).
# Each core projects Q/K/V for its own 512 tokens in bf16; K^T and V (with a
# softmax ones-column) are packed into one buffer and AllGather'd within the
# 4-core batch group. Attention runs against the gathered full-sequence K/V.
# All matmul operands are bf16 (PSUM accumulation stays f32); LayerNorm,
# softmax normalization and residuals are f32. Weights are pre-transposed on
# the host into [128, KC, N] layouts so every DMA moves >=1KB contiguous runs.
# LayerNorm big elementwise ops are spread across DVE/ACT/GpSimd so the four
# per-token-block chains pipeline instead of serializing on DVE.
import numpy as np

B, S, D = 2, 2048, 1024
H, DK, DVH, DFF = 16, 64, 64, 4096
TOK = S // 4
NP = H // 2
KTILES = S // 128
KC = D // 128
MH = DFF // 128
EPS = 1e-5
KCOLS = NP * 512          # 4096 K^T columns per partition row
VCOLS = 4 * H * 65        # 4160 V columns (64 values + 1 ones per head)
BLK = KCOLS + VCOLS       # 8256

_CACHE = {}


def _build():
    import concourse.mybir as mybir
    import concourse.tile as tile
    from concourse import bacc

    f32, f32r, bf16 = mybir.dt.float32, mybir.dt.float32r, mybir.dt.bfloat16
    Exp = mybir.ActivationFunctionType.Exp
    Relu = mybir.ActivationFunctionType.Relu
    Ident = mybir.ActivationFunctionType.Identity
    AX = mybir.AxisListType.X
    Alu = mybir.AluOpType

    nc = bacc.Bacc("TRN2", target_bir_lowering=False, debug=False, num_devices=8)

    xb_d = nc.dram_tensor("xb", [TOK, D], f32, kind="ExternalInput")
    wq_d = nc.dram_tensor("wq_r", [128, KC, H * DK], bf16, kind="ExternalInput")
    wk_d = nc.dram_tensor("wk_r", [128, KC, H * DK], bf16, kind="ExternalInput")
    wv_d = nc.dram_tensor("wv_r", [128, KC, H * DVH], bf16, kind="ExternalInput")
    wo_d = nc.dram_tensor("wo_r", [128, KC, D], bf16, kind="ExternalInput")
    w1_d = nc.dram_tensor("w1_r", [128, KC, DFF], bf16, kind="ExternalInput")
    w2_d = nc.dram_tensor("w2_r", [128, MH, D], bf16, kind="ExternalInput")
    b1c_d = nc.dram_tensor("b1c", [128, MH], f32, kind="ExternalInput")
    b2r_d = nc.dram_tensor("b2r", [1, D], f32r, kind="ExternalInput")
    g1bc_d = nc.dram_tensor("g1bc", [128, D], f32, kind="ExternalInput")
    h1bc_d = nc.dram_tensor("h1bc", [128, D], f32, kind="ExternalInput")
    g2bc_d = nc.dram_tensor("g2bc", [128, D], f32, kind="ExternalInput")
    h2bc_d = nc.dram_tensor("h2bc", [128, D], f32, kind="ExternalInput")
    ident_d = nc.dram_tensor("ident", [128, 128], f32, kind="ExternalInput")
    ones64_d = nc.dram_tensor("ones64", [1, 64], f32r, kind="ExternalInput")
    ones128_d = nc.dram_tensor("ones128", [1, 128], f32r, kind="ExternalInput")
    y_d = nc.dram_tensor("y_part", [TOK, D], f32, kind="ExternalOutput")

    def ln_quad(pool, ts, gbc, hbc, outs, finals=None):
        # LayerNorm over the free axis for several [128, D] tiles (inputs may
        # live in PSUM — stats and xa read them directly, no staging copy).
        # Issued in wavefront (skewed) order: engines are strict FIFO, so
        # chain i stage s follows chain i-1 stage s and the chains pipeline
        # across DVE/ACT/GpSimd instead of serializing.
        n = len(ts)
        mk = lambda tag, shape: [
            pool.tile(shape, f32, tag=f"{tag}{i}", name=f"{tag}{i}")
            for i in range(n)
        ]
        sums = mk("ln_sums", [128, 1])
        sq = mk("ln_sq", [128, D])
        ssq = mk("ln_ssq", [128, 1])
        s2 = mk("ln_s2", [128, 1])
        var0 = mk("ln_var0", [128, 1])
        s2b = mk("ln_s2b", [128, 1])
        var = mk("ln_var", [128, 1])
        sd = mk("ln_sd", [128, 1])
        rv = mk("ln_rv", [128, 1])
        nmr = mk("ln_nmr", [128, 1])
        nmr2 = mk("ln_nmr2", [128, 1])
        xa = mk("ln_xa", [128, D])

        def s_stats(i):
            nc.vector.reduce_sum(sums[i][:], ts[i], axis=AX)
            nc.scalar.activation(
                sq[i][:], ts[i], mybir.ActivationFunctionType.Square,
                accum_out=ssq[i][:],
            )

        def s_smalls(i):
            nc.vector.tensor_mul(s2[i][:], sums[i][:], sums[i][:])
            nc.vector.tensor_scalar(
                out=var0[i][:], in0=ssq[i][:], scalar1=1.0 / D, scalar2=EPS,
                op0=Alu.mult, op1=Alu.add,
            )
            nc.vector.tensor_scalar_mul(s2b[i][:], s2[i][:], 1.0 / (D * D))
            nc.vector.tensor_sub(var[i][:], var0[i][:], s2b[i][:])

        def s_sqrt(i):
            nc.scalar.activation(sd[i][:], var[i][:],
                                 mybir.ActivationFunctionType.Sqrt)

        def s_recip(i):
            nc.vector.reciprocal(rv[i][:], sd[i][:])
            nc.vector.tensor_mul(nmr[i][:], sums[i][:], rv[i][:])
            nc.vector.tensor_scalar_mul(nmr2[i][:], nmr[i][:], -1.0 / D)

        def s_xa(i):
            nc.scalar.activation(xa[i][:], ts[i], Ident,
                                 bias=nmr2[i][:], scale=rv[i][:])

        def s_mul(i):
            nc.vector.tensor_mul(sq[i][:], xa[i][:], gbc[:])

        def s_add(i):
            nc.gpsimd.tensor_add(outs[i], sq[i][:], hbc[:])

        stages = [s_stats, s_smalls, s_sqrt, s_recip, s_xa, s_mul, s_add]
        if finals is not None:
            stages = stages + [finals]
        m = len(stages)
        for wave in range(n + m - 1):
            for i in range(n):
                s = wave - i
                if 0 <= s < m:
                    stages[s](i)

    with tile.TileContext(nc) as tc:
        with (
            tc.tile_pool(name="const", bufs=1) as cpool,
            tc.tile_pool(name="dram", bufs=1, space="DRAM") as dram,
        ):
            # x + ident first: they gate the transposes that gate everything
            xfull = cpool.tile([128, 4, D], f32)
            nc.sync.dma_start(
                xfull[:], xb_d.ap().rearrange("(a p) d -> p a d", p=128)
            )
            ident = cpool.tile([128, 128], f32)
            nc.scalar.dma_start(ident[:], ident_d.ap())
            ones64 = cpool.tile([1, 64], f32r)
            nc.scalar.dma_start(ones64[:], ones64_d.ap())
            ones128 = cpool.tile([1, 128], f32r)
            nc.scalar.dma_start(ones128[:], ones128_d.ap())
            b2r = cpool.tile([1, D], f32r)
            nc.gpsimd.dma_start(b2r[:], b2r_d.ap())
            g1bc = cpool.tile([128, D], f32)
            nc.gpsimd.dma_start(g1bc[:], g1bc_d.ap())
            h1bc = cpool.tile([128, D], f32)
            nc.gpsimd.dma_start(h1bc[:], h1bc_d.ap())
            g2bc = cpool.tile([128, D], f32)
            nc.gpsimd.dma_start(g2bc[:], g2bc_d.ap())
            h2bc = cpool.tile([128, D], f32)
            nc.gpsimd.dma_start(h2bc[:], h2bc_d.ap())
            b1c = cpool.tile([128, MH], f32)
            nc.gpsimd.dma_start(b1c[:], b1c_d.ap())
            o_norm = cpool.tile([128, NP, TOK], bf16)
            # bf16 identity + bf16 x copies feed the matmul residual adds
            identb = cpool.tile([128, 128], bf16)
            xfb = cpool.tile([128, 4, D], bf16)

            comb_in = dram.tile([128, BLK], bf16)
            comb_out = dram.tile([4, 128, BLK], bf16)

            with (
                tc.tile_pool(name="mid", bufs=1) as midp,
                tc.tile_pool(name="qp", bufs=1) as qp,
            ):
                qT = qp.tile([128, NP, TOK], bf16)
                x1 = midp.tile([128, 4, D], f32)
                x1T = midp.tile([128, KC, TOK], bf16)

                # ---- Phase A: transpose own x -> xT (bf16)
                with tc.tile_pool(name="xtp", bufs=1) as xtp:
                    xT = xtp.tile([128, KC, TOK], bf16)
                    with tc.tile_pool(name="ps_tr", bufs=2, space="PSUM") as ps_tr:
                        for dc in range(KC):
                            ps = ps_tr.tile([128, 4, 128], f32, tag="trp", name="trp")
                            for a in range(4):
                                nc.tensor.transpose(
                                    ps[:, a, :],
                                    xfull[:, a, dc * 128:(dc + 1) * 128],
                                    ident[:],
                                )
                            nc.vector.tensor_copy(
                                xT[:, dc, :], ps[:].rearrange("p a t -> p (a t)")
                            )

                    # ---- Phase B: Q, K, V projections on own tokens (bf16)
                    with (
                        tc.tile_pool(name="wqk", bufs=1) as wqk,
                        tc.tile_pool(name="stg", bufs=4) as stg,
                        tc.tile_pool(name="ps_q", bufs=4, space="PSUM") as ps_q,
                    ):
                        wk_sb = wqk.tile([128, KC, H * DK], bf16, name="wk_sb")
                        nc.scalar.dma_start(wk_sb[:], wk_d.ap())
                        wv_sb = wqk.tile([128, KC, H * DVH], bf16, name="wv_sb")
                        nc.sync.dma_start(wv_sb[:], wv_d.ap())
                        wq_sb = wqk.tile([128, KC, H * DK], bf16, name="wq_sb")

                        # K first (feeds the collective)
                        for p in range(NP):
                            ps = ps_q.tile([128, 512], f32, tag="psq", name="psq")
                            for kc in range(KC):
                                nc.tensor.matmul(
                                    ps[:],
                                    wk_sb[:, kc, p * 128:(p + 1) * 128],
                                    xT[:, kc, :],
                                    start=(kc == 0), stop=(kc == KC - 1),
                                )
                            st = stg.tile([128, 512], bf16, tag="kst", name="kst")
                            nc.vector.tensor_copy(st[:], ps[:])
                            nc.sync.dma_start(
                                comb_in[:, p * 512:(p + 1) * 512], st[:]
                            )
                        # V with interleaved softmax-ones column
                        vstg = wqk.tile([128, 4, H, 65], bf16, name="vstg")
                        nc.gpsimd.memset(vstg[:, :, :, 64:65], 1.0)
                        for mtk in range(4):
                            for ncc in range(2):
                                ps = ps_q.tile([128, 512], f32, tag="psq", name="psq")
                                for kc in range(KC):
                                    nc.tensor.matmul(
                                        ps[:],
                                        xT[:, kc, mtk * 128:(mtk + 1) * 128],
                                        wv_sb[:, kc, ncc * 512:(ncc + 1) * 512],
                                        start=(kc == 0), stop=(kc == KC - 1),
                                    )
                                nc.scalar.activation(
                                    vstg[:, mtk, ncc * 8:(ncc + 1) * 8, 0:64],
                                    ps[:].rearrange("p (h v) -> p h v", h=8),
                                    Ident,
                                )
                            nc.scalar.dma_start(
                                comb_in[:, KCOLS + mtk * (H * 65):
                                        KCOLS + (mtk + 1) * (H * 65)].rearrange(
                                    "p (h v) -> p h v", h=H
                                ),
                                vstg[:, mtk, :, :],
                            )
                        nc.gpsimd.collective_compute(
                            "AllGather",
                            Alu.bypass,
                            ins=[comb_in.opt()],
                            outs=[comb_out.opt()],
                            replica_groups=[[0, 1, 2, 3], [4, 5, 6, 7]],
                        )
                        # dep-free work below overlaps the collective
                        nc.sync.dma_start(wq_sb[:], wq_d.ap())
                        nc.vector.tensor_copy(identb[:], ident[:])
                        nc.vector.tensor_copy(
                            xfb[:].rearrange("p a d -> p (a d)"),
                            xfull[:].rearrange("p a d -> p (a d)"),
                        )
                        # Q projection overlaps the collective
                        for p in range(NP):
                            ps = ps_q.tile([128, 512], f32, tag="psq", name="psq")
                            for kc in range(KC):
                                nc.tensor.matmul(
                                    ps[:],
                                    wq_sb[:, kc, p * 128:(p + 1) * 128],
                                    xT[:, kc, :],
                                    start=(kc == 0), stop=(kc == KC - 1),
                                )
                            if p % 2 == 0:
                                nc.vector.tensor_copy(qT[:, p, :], ps[:])
                            else:
                                nc.scalar.activation(qT[:, p, :], ps[:], Ident)

                # ---- Phase C: attention against gathered K/V
                wop_cm = tc.tile_pool(name="wop", bufs=1)
                wop = wop_cm.__enter__()
                wosb = wop.tile([128, KC, D], bf16)
                nc.gpsimd.dma_start(wosb[:], wo_d.ap())
                with (
                    tc.tile_pool(name="vsb", bufs=1) as vsbp,
                    tc.tile_pool(name="at", bufs=6) as atpool,
                    tc.tile_pool(name="rec", bufs=3) as recpool,
                    tc.tile_pool(name="ps_s", bufs=2, space="PSUM") as ps_s,
                    tc.tile_pool(name="ps_o", bufs=3, space="PSUM") as ps_o,
                    tc.tile_pool(name="ps_r", bufs=1, space="PSUM") as ps_r,
                ):
                    # prefetch ALL gathered K^T and V on parallel queues;
                    # rank-0 slices first so kt 0..3 unblock immediately
                    ktp_all = vsbp.tile([128, NP, 4, 512], bf16)
                    v_sb = vsbp.tile([128, KTILES, H, 65], bf16)
                    # tiny p=0 K slices first so scores p0 starts at once,
                    # then rank-0 V (first attnV), then the bulk
                    for r in range(4):
                        (nc.sync if r % 2 == 0 else nc.scalar).dma_start(
                            ktp_all[:, 0, r, :], comb_out[r, :, 0:512]
                        )
                    nc.gpsimd.dma_start(
                        v_sb[:, 0:4, :, :],
                        comb_out[0, :, KCOLS:].rearrange(
                            "p (t h v) -> p t h v", t=4, h=H
                        ),
                    )
                    kq = [nc.sync, nc.scalar, nc.gpsimd, nc.sync]
                    for r in range(4):
                        kq[r].dma_start(
                            ktp_all[:, 1:NP, r, :],
                            comb_out[r, :, 512:KCOLS].rearrange(
                                "p (np t) -> p np t", np=NP - 1
                            ),
                        )
                        if r > 0:
                            kq[(r + 1) % 4].dma_start(
                                v_sb[:, 4 * r:4 * (r + 1), :, :],
                                comb_out[r, :, KCOLS:].rearrange(
                                    "p (t h v) -> p t h v", t=4, h=H
                                ),
                            )
                    for p in range(NP):
                        po = [
                            ps_o.tile([65, TOK], f32, tag="po", name=f"po{p}_{hh}")
                            for hh in range(2)
                        ]
                        for g in range(8):
                            for hh in range(2):
                                sT = ps_s.tile([128, 2, 512], f32, tag="sT", name="sT")
                                for j in range(2):
                                    kt = 2 * g + j
                                    nc.tensor.matmul(
                                        sT[:, j, :],
                                        ktp_all[hh * 64:(hh + 1) * 64, p, :, :]
                                        .rearrange("p r t -> p (r t)")[
                                            :, kt * 128:(kt + 1) * 128],
                                        qT[hh * 64:(hh + 1) * 64, p, :],
                                        tile_position=(hh * 64, 0),
                                    )
                                at = atpool.tile([128, 2, 512], bf16, tag="at", name="at")
                                nc.scalar.activation(at[:], sT[:], Exp, scale=0.125)
                                for j in range(2):
                                    kt = 2 * g + j
                                    nc.tensor.matmul(
                                        po[hh][:],
                                        v_sb[:, kt, 2 * p + hh, :],
                                        at[:, j, :],
                                        start=(kt == 0), stop=(kt == KTILES - 1),
                                    )
                        for hh in range(2):
                            rec = recpool.tile([1, TOK], f32r, tag="rec", name="rec")
                            with nc.allow_low_precision(reason="f32r"):
                                nc.vector.reciprocal(rec[:], po[hh][64:65, :])
                            rp = ps_r.tile([64, TOK], f32, tag="rp", name="rp")
                            nc.tensor.matmul(rp[:], ones64[:], rec[:])
                            rsb = recpool.tile([64, TOK], f32, tag="rsb", name="rsb")
                            nc.vector.tensor_copy(rsb[:], rp[:])
                            nc.vector.tensor_mul(
                                o_norm[hh * 64:(hh + 1) * 64, p, :],
                                po[hh][0:64, :],
                                rsb[:],
                            )

                # ---- Phase D+E head: Wo + residual + LN1 + x1T + FFN1
                # (residual adds ride the PSUM accumulation as identity
                # matmuls; LayerNorm stats read PSUM directly)
                with (
                    tc.tile_pool(name="lnd", bufs=1) as lnd,
                    tc.tile_pool(name="ht", bufs=1) as htp,
                ):
                    hT = htp.tile([128, MH, TOK], bf16)
                    psos = []
                    with tc.tile_pool(name="ps_wo", bufs=1, space="PSUM") as ps_wo:
                        for mt in range(4):
                            pso = ps_wo.tile([128, D], f32, tag=f"pso{mt}",
                                             name=f"pso{mt}")
                            for kc in range(KC):
                                for ncc in range(2):
                                    nc.tensor.matmul(
                                        pso[:, ncc * 512:(ncc + 1) * 512],
                                        o_norm[:, kc, mt * 128:(mt + 1) * 128],
                                        wosb[:, kc, ncc * 512:(ncc + 1) * 512],
                                        start=(kc == 0), stop=False,
                                    )
                            for ncc in range(2):
                                nc.tensor.matmul(
                                    pso[:, ncc * 512:(ncc + 1) * 512],
                                    identb[:],
                                    xfb[:, mt, ncc * 512:(ncc + 1) * 512],
                                    start=False, stop=(ncc == 1),
                                )
                            psos.append(pso)
                        ln_quad(lnd, [pso[:] for pso in psos], g1bc, h1bc,
                                [x1[:, mt, :] for mt in range(4)])
                    with tc.tile_pool(name="ps_t2", bufs=2, space="PSUM") as ps_t2:
                        for mt in range(4):
                            for half in range(2):
                                ps = ps_t2.tile([128, 4, 128], f32, tag="trp2",
                                                name="trp2")
                                for q in range(4):
                                    dc = half * 4 + q
                                    nc.tensor.transpose(
                                        ps[:, q, :],
                                        x1[:, mt, dc * 128:(dc + 1) * 128],
                                        ident[:],
                                    )
                                nc.vector.tensor_copy(
                                    x1T[:, half * 4:(half + 1) * 4,
                                        mt * 128:(mt + 1) * 128],
                                    ps[:],
                                )

                    # FFN1, streaming W1 in mh-pairs; also cast x1 -> bf16
                    # for the FFN2 residual matmul
                    with (
                        tc.tile_pool(name="w1p", bufs=4) as w1p,
                        tc.tile_pool(name="ps_f1", bufs=2, space="PSUM") as ps_f1,
                    ):
                        x1b = midp.tile([128, 4, D], bf16)
                        nc.vector.tensor_copy(
                            x1b[:].rearrange("p a d -> p (a d)"),
                            x1[:].rearrange("p a d -> p (a d)"),
                        )
                        w1t = None
                        for mh in range(MH):
                            if mh % 2 == 0:
                                w1t = w1p.tile([128, KC, 256], bf16, tag="w1t",
                                               name="w1t")
                                (nc.sync if mh % 4 == 0 else nc.scalar).dma_start(
                                    w1t[:],
                                    w1_d.ap()[:, :, mh * 128:(mh + 2) * 128],
                                )
                            ps = ps_f1.tile([128, 512], f32, tag="psf1", name="psf1")
                            for dc in range(KC):
                                nc.tensor.matmul(
                                    ps[:],
                                    w1t[:, dc, (mh % 2) * 128:(mh % 2 + 1) * 128],
                                    x1T[:, dc, :],
                                    start=(dc == 0), stop=(dc == KC - 1),
                                )
                            if mh % 2 == 0:
                                nc.scalar.activation(
                                    hT[:, mh, :], ps[:], Relu,
                                    bias=b1c[:, mh:mh + 1],
                                )
                            else:
                                nc.vector.tensor_scalar(
                                    out=hT[:, mh, :], in0=ps[:],
                                    scalar1=b1c[:, mh:mh + 1], scalar2=0.0,
                                    op0=Alu.add, op1=Alu.max,
                                )

                    # ---- Phase E tail: FFN2 (+bias+residual in PSUM),
                    # LayerNorm straight out of PSUM, store
                    with (
                        tc.tile_pool(name="w2p", bufs=4) as w2p,
                        tc.tile_pool(name="ps_f2", bufs=1, space="PSUM") as ps_f2,
                        tc.tile_pool(name="outp", bufs=1) as outp,
                    ):
                        psy = [
                            ps_f2.tile([128, D], f32, tag=f"py{mt}", name=f"py{mt}")
                            for mt in range(4)
                        ]
                        for mh in range(MH):
                            w2t = w2p.tile([128, D], bf16, tag="w2t", name="w2t")
                            (nc.sync if mh % 2 == 0 else nc.scalar).dma_start(
                                w2t[:], w2_d.ap()[:, mh, :]
                            )
                            for mt in range(4):
                                for ncc in range(2):
                                    nc.tensor.matmul(
                                        psy[mt][:, ncc * 512:(ncc + 1) * 512],
                                        hT[:, mh, mt * 128:(mt + 1) * 128],
                                        w2t[:, ncc * 512:(ncc + 1) * 512],
                                        start=(mh == 0), stop=False,
                                    )
                        for mt in range(4):
                            for ncc in range(2):
                                nc.tensor.matmul(
                                    psy[mt][:, ncc * 512:(ncc + 1) * 512],
                                    ones128[:],
                                    b2r[:, ncc * 512:(ncc + 1) * 512],
                                    start=False, stop=False,
                                )
                                nc.tensor.matmul(
                                    psy[mt][:, ncc * 512:(ncc + 1) * 512],
                                    identb[:],
                                    x1b[:, mt, ncc * 512:(ncc + 1) * 512],
                                    start=False, stop=(ncc == 1),
                                )
                        t2s = [
                            outp.tile([128, D], f32, tag=f"t2{mt}", name=f"t2{mt}")
                            for mt in range(4)
                        ]

                        def store_final(mt):
                            (nc.sync if mt % 2 == 0 else nc.scalar).dma_start(
                                y_d.ap()[mt * 128:(mt + 1) * 128, :], t2s[mt][:]
                            )
                        ln_quad(lnd, [p[:] for p in psy], g2bc, h2bc,
                                [t[:] for t in t2s], finals=store_final)
                wop_cm.__exit__(None, None, None)
    nc.compile()
    return nc


def _in_maps(x, Wq, Wk, Wv, Wo, ln1_g, ln1_b, W1, b1, W2, b2, ln2_g, ln2_b):
    import ml_dtypes

    bf16 = ml_dtypes.bfloat16
    x = np.ascontiguousarray(np.asarray(x, np.float32))

    def to_sb(w, ncols):
        # [D_in, N] -> [128, D_in//128, N] partition-major layout, bf16
        w = np.asarray(w, np.float32).reshape(-1, 128, ncols).transpose(1, 0, 2)
        return np.ascontiguousarray(w.astype(bf16))

    wq2 = np.asarray(Wq, np.float32).transpose(1, 0, 2).reshape(D, H * DK)
    wk2 = np.asarray(Wk, np.float32).transpose(1, 0, 2).reshape(D, H * DK)
    wv2 = np.asarray(Wv, np.float32).transpose(1, 0, 2).reshape(D, H * DVH)
    bcast = lambda v: np.ascontiguousarray(
        np.broadcast_to(np.asarray(v, np.float32), (128, D))
    )
    common = {
        "wq_r": to_sb(wq2, H * DK), "wk_r": to_sb(wk2, H * DK),
        "wv_r": to_sb(wv2, H * DVH),
        "wo_r": to_sb(np.asarray(Wo, np.float32), D),
        "w1_r": to_sb(np.asarray(W1, np.float32), DFF),
        "w2_r": to_sb(np.asarray(W2, np.float32), D),
        "b1c": np.ascontiguousarray(np.asarray(b1, np.float32).reshape(MH, 128).T),
        "b2r": np.ascontiguousarray(np.asarray(b2, np.float32).reshape(1, D)),
        "g1bc": bcast(ln1_g), "h1bc": bcast(ln1_b),
        "g2bc": bcast(ln2_g), "h2bc": bcast(ln2_b),
        "ident": np.eye(128, dtype=np.float32),
        "ones64": np.ones((1, 64), np.float32),
        "ones128": np.ones((1, 128), np.float32),
    }
    in_maps = []
    for c in range(8):
        b, q0 = c // 4, TOK * (c % 4)
        m = dict(common)
        m["xb"] = np.ascontiguousarray(x[b, q0:q0 + TOK, :])
        in_maps.append(m)
    return in_maps


def kernel(x, Wq, Wk, Wv, Wo, ln1_g, ln1_b, W1, b1, W2, b2, ln2_g, ln2_b):
    from concourse.bass_utils import run_bass_kernel_spmd

    if "nc" not in _CACHE:
        _CACHE["nc"] = _build()
    nc = _CACHE["nc"]
    in_maps = _in_maps(x, Wq, Wk, Wv, Wo, ln1_g, ln1_b, W1, b1, W2, b2, ln2_g, ln2_b)
    res = run_bass_kernel_spmd(nc, in_maps, core_ids=list(range(8)))
    out = np.empty((B, S, D), np.float32)
    for c in range(8):
        b, q0 = c // 4, TOK * (c % 4)
        out[b, q0:q0 + TOK, :] = res.results[c]["y_part"]
    return out


# revision 23
# speedup vs baseline: 1.0361x; 1.0361x over previous
# v5.2: single combined K+V AllGather in bf16 over 4-rank replica groups.
# Data-parallel over tokens (8 cores x 512 tokens, 4 cores per batch).
# Each core projects Q/K/V for its own 512 tokens in bf16; K^T and V (with a
# softmax ones-column) are packed into one buffer and AllGather'd within the
# 4-core batch group. Attention runs against the gathered full-sequence K/V.
# All matmul operands are bf16 (PSUM accumulation stays f32); LayerNorm,
# softmax normalization and residuals are f32. Weights are pre-transposed on
# the host into [128, KC, N] layouts so every DMA moves >=1KB contiguous runs.
# LayerNorm big elementwise ops are spread across DVE/ACT/GpSimd so the four
# per-token-block chains pipeline instead of serializing on DVE.
import numpy as np

B, S, D = 2, 2048, 1024
H, DK, DVH, DFF = 16, 64, 64, 4096
TOK = S // 4
NP = H // 2
KTILES = S // 128
KC = D // 128
MH = DFF // 128
EPS = 1e-5
KCOLS = NP * 512          # 4096 K^T columns per partition row
VCOLS = 4 * H * 65        # 4160 V columns (64 values + 1 ones per head)
BLK = KCOLS + VCOLS       # 8256

_CACHE = {}


def _build():
    import concourse.mybir as mybir
    import concourse.tile as tile
    from concourse import bacc

    f32, f32r, bf16 = mybir.dt.float32, mybir.dt.float32r, mybir.dt.bfloat16
    Exp = mybir.ActivationFunctionType.Exp
    Relu = mybir.ActivationFunctionType.Relu
    Ident = mybir.ActivationFunctionType.Identity
    AX = mybir.AxisListType.X
    Alu = mybir.AluOpType

    nc = bacc.Bacc("TRN2", target_bir_lowering=False, debug=False, num_devices=8)

    xb_d = nc.dram_tensor("xb", [TOK, D], f32, kind="ExternalInput")
    wq_d = nc.dram_tensor("wq_r", [128, NP, KC, 128], bf16, kind="ExternalInput")
    wk_d = nc.dram_tensor("wk_r", [128, NP, KC, 128], bf16, kind="ExternalInput")
    wv_d = nc.dram_tensor("wv_r", [128, 2, KC, 512], bf16, kind="ExternalInput")
    wo_d = nc.dram_tensor("wo_r", [128, KC, D], bf16, kind="ExternalInput")
    w1_d = nc.dram_tensor("w1_r", [128, KC, DFF], bf16, kind="ExternalInput")
    w2_d = nc.dram_tensor("w2_r", [128, MH, D], bf16, kind="ExternalInput")
    b1c_d = nc.dram_tensor("b1c", [128, MH], f32, kind="ExternalInput")
    b2r_d = nc.dram_tensor("b2r", [1, D], f32r, kind="ExternalInput")
    g1bc_d = nc.dram_tensor("g1bc", [128, D], f32, kind="ExternalInput")
    h1bc_d = nc.dram_tensor("h1bc", [128, D], f32, kind="ExternalInput")
    g2bc_d = nc.dram_tensor("g2bc", [128, D], f32, kind="ExternalInput")
    h2bc_d = nc.dram_tensor("h2bc", [128, D], f32, kind="ExternalInput")
    ident_d = nc.dram_tensor("ident", [128, 128], f32, kind="ExternalInput")
    ones64_d = nc.dram_tensor("ones64", [1, 64], f32r, kind="ExternalInput")
    ones128_d = nc.dram_tensor("ones128", [1, 128], f32r, kind="ExternalInput")
    y_d = nc.dram_tensor("y_part", [TOK, D], f32, kind="ExternalOutput")

    def ln_quad(pool, ts, gbc, hbc, outs, finals=None):
        # LayerNorm over the free axis for several [128, D] tiles (inputs may
        # live in PSUM — stats and xa read them directly, no staging copy).
        # Issued in wavefront (skewed) order: engines are strict FIFO, so
        # chain i stage s follows chain i-1 stage s and the chains pipeline
        # across DVE/ACT/GpSimd instead of serializing.
        n = len(ts)
        mk = lambda tag, shape: [
            pool.tile(shape, f32, tag=f"{tag}{i}", name=f"{tag}{i}")
            for i in range(n)
        ]
        sums = mk("ln_sums", [128, 1])
        sq = mk("ln_sq", [128, D])
        ssq = mk("ln_ssq", [128, 1])
        s2 = mk("ln_s2", [128, 1])
        var0 = mk("ln_var0", [128, 1])
        s2b = mk("ln_s2b", [128, 1])
        var = mk("ln_var", [128, 1])
        sd = mk("ln_sd", [128, 1])
        rv = mk("ln_rv", [128, 1])
        nmr = mk("ln_nmr", [128, 1])
        nmr2 = mk("ln_nmr2", [128, 1])
        xa = mk("ln_xa", [128, D])

        def s_stats(i):
            nc.vector.reduce_sum(sums[i][:], ts[i], axis=AX)
            nc.scalar.activation(
                sq[i][:], ts[i], mybir.ActivationFunctionType.Square,
                accum_out=ssq[i][:],
            )

        def s_smalls(i):
            nc.vector.tensor_mul(s2[i][:], sums[i][:], sums[i][:])
            nc.vector.tensor_scalar(
                out=var0[i][:], in0=ssq[i][:], scalar1=1.0 / D, scalar2=EPS,
                op0=Alu.mult, op1=Alu.add,
            )
            nc.vector.tensor_scalar_mul(s2b[i][:], s2[i][:], 1.0 / (D * D))
            nc.vector.tensor_sub(var[i][:], var0[i][:], s2b[i][:])

        def s_sqrt(i):
            nc.scalar.activation(sd[i][:], var[i][:],
                                 mybir.ActivationFunctionType.Sqrt)

        def s_recip(i):
            nc.vector.reciprocal(rv[i][:], sd[i][:])
            nc.vector.tensor_mul(nmr[i][:], sums[i][:], rv[i][:])
            nc.vector.tensor_scalar_mul(nmr2[i][:], nmr[i][:], -1.0 / D)

        def s_xa(i):
            nc.scalar.activation(xa[i][:], ts[i], Ident,
                                 bias=nmr2[i][:], scale=rv[i][:])

        def s_mul(i):
            nc.vector.tensor_mul(sq[i][:], xa[i][:], gbc[:])

        def s_add(i):
            nc.gpsimd.tensor_add(outs[i], sq[i][:], hbc[:])

        stages = [s_stats, s_smalls, s_sqrt, s_recip, s_xa, s_mul, s_add]
        if finals is not None:
            stages = stages + [finals]
        m = len(stages)
        for wave in range(n + m - 1):
            for i in range(n):
                s = wave - i
                if 0 <= s < m:
                    stages[s](i)

    with tile.TileContext(nc) as tc:
        with (
            tc.tile_pool(name="const", bufs=1) as cpool,
            tc.tile_pool(name="dram", bufs=1, space="DRAM") as dram,
        ):
            # x + ident first: they gate the transposes that gate everything
            xfull = cpool.tile([128, 4, D], f32)
            for a in range(4):
                nc.sync.dma_start(
                    xfull[:, a, :], xb_d.ap()[a * 128:(a + 1) * 128, :]
                )
            ident = cpool.tile([128, 128], f32)
            nc.scalar.dma_start(ident[:], ident_d.ap())
            ones64 = cpool.tile([1, 64], f32r)
            nc.scalar.dma_start(ones64[:], ones64_d.ap())
            ones128 = cpool.tile([1, 128], f32r)
            nc.scalar.dma_start(ones128[:], ones128_d.ap())
            b2r = cpool.tile([1, D], f32r)
            g1bc = cpool.tile([128, D], f32)
            h1bc = cpool.tile([128, D], f32)
            g2bc = cpool.tile([128, D], f32)
            h2bc = cpool.tile([128, D], f32)
            b1c = cpool.tile([128, MH], f32)
            o_norm = cpool.tile([128, NP, TOK], bf16)
            # bf16 identity + bf16 x copies feed the matmul residual adds
            identb = cpool.tile([128, 128], bf16)
            xfb = cpool.tile([128, 4, D], bf16)

            comb_in = dram.tile([128, BLK], bf16)
            comb_out = dram.tile([4, 128, BLK], bf16)

            with (
                tc.tile_pool(name="mid", bufs=1) as midp,
                tc.tile_pool(name="qp", bufs=1) as qp,
            ):
                qT = qp.tile([128, NP, TOK], bf16)
                x1 = midp.tile([128, 4, D], f32)
                x1T = midp.tile([128, KC, TOK], bf16)

                # ---- Phase A: transpose own x -> xT (bf16)
                with tc.tile_pool(name="xtp", bufs=1) as xtp:
                    xT = xtp.tile([128, KC, TOK], bf16)
                    with tc.tile_pool(name="ps_tr", bufs=2, space="PSUM") as ps_tr:
                        for dc in range(KC):
                            ps = ps_tr.tile([128, 4, 128], f32, tag="trp", name="trp")
                            for a in range(4):
                                nc.tensor.transpose(
                                    ps[:, a, :],
                                    xfull[:, a, dc * 128:(dc + 1) * 128],
                                    ident[:],
                                )
                            nc.vector.tensor_copy(
                                xT[:, dc, :], ps[:].rearrange("p a t -> p (a t)")
                            )

                    # ---- Phase B: Q, K, V projections on own tokens (bf16)
                    with (
                        tc.tile_pool(name="wqk", bufs=1) as wqk,
                        tc.tile_pool(name="stg", bufs=4) as stg,
                        tc.tile_pool(name="ps_q", bufs=4, space="PSUM") as ps_q,
                    ):
                        wk_sb = wqk.tile([128, NP, KC, 128], bf16, name="wk_sb")
                        for p in range(NP):
                            nc.scalar.dma_start(
                                wk_sb[:, p, :, :], wk_d.ap()[:, p, :, :]
                            )
                        wv_sb = wqk.tile([128, 2, KC, 512], bf16, name="wv_sb")
                        for ncc in range(2):
                            nc.sync.dma_start(
                                wv_sb[:, ncc, :, :], wv_d.ap()[:, ncc, :, :]
                            )
                        wq_sb = wqk.tile([128, NP, KC, 128], bf16, name="wq_sb")

                        # K first (feeds the collective)
                        for p in range(NP):
                            ps = ps_q.tile([128, 512], f32, tag="psq", name="psq")
                            for kc in range(KC):
                                nc.tensor.matmul(
                                    ps[:],
                                    wk_sb[:, p, kc, :],
                                    xT[:, kc, :],
                                    start=(kc == 0), stop=(kc == KC - 1),
                                )
                            st = stg.tile([128, 512], bf16, tag="kst", name="kst")
                            nc.vector.tensor_copy(st[:], ps[:])
                            nc.sync.dma_start(
                                comb_in[:, p * 512:(p + 1) * 512], st[:]
                            )
                        # V with interleaved softmax-ones column
                        vstg = wqk.tile([128, 4, H, 65], bf16, name="vstg")
                        nc.gpsimd.memset(vstg[:, :, :, 64:65], 1.0)
                        for mtk in range(4):
                            for ncc in range(2):
                                ps = ps_q.tile([128, 512], f32, tag="psq", name="psq")
                                for kc in range(KC):
                                    nc.tensor.matmul(
                                        ps[:],
                                        xT[:, kc, mtk * 128:(mtk + 1) * 128],
                                        wv_sb[:, ncc, kc, :],
                                        start=(kc == 0), stop=(kc == KC - 1),
                                    )
                                nc.scalar.activation(
                                    vstg[:, mtk, ncc * 8:(ncc + 1) * 8, 0:64],
                                    ps[:].rearrange("p (h v) -> p h v", h=8),
                                    Ident,
                                )
                            nc.scalar.dma_start(
                                comb_in[:, KCOLS + mtk * (H * 65):
                                        KCOLS + (mtk + 1) * (H * 65)].rearrange(
                                    "p (h v) -> p h v", h=H
                                ),
                                vstg[:, mtk, :, :],
                            )
                        nc.gpsimd.collective_compute(
                            "AllGather",
                            Alu.bypass,
                            ins=[comb_in.opt()],
                            outs=[comb_out.opt()],
                            replica_groups=[[0, 1, 2, 3], [4, 5, 6, 7]],
                        )
                        # dep-free work below overlaps the collective
                        nc.sync.dma_start(wq_sb[:], wq_d.ap())
                        nc.gpsimd.dma_start(b2r[:], b2r_d.ap())
                        nc.gpsimd.dma_start(g1bc[:], g1bc_d.ap())
                        nc.gpsimd.dma_start(h1bc[:], h1bc_d.ap())
                        nc.gpsimd.dma_start(g2bc[:], g2bc_d.ap())
                        nc.gpsimd.dma_start(h2bc[:], h2bc_d.ap())
                        nc.gpsimd.dma_start(b1c[:], b1c_d.ap())
                        nc.vector.tensor_copy(identb[:], ident[:])
                        nc.vector.tensor_copy(
                            xfb[:].rearrange("p a d -> p (a d)"),
                            xfull[:].rearrange("p a d -> p (a d)"),
                        )
                        # Q projection overlaps the collective
                        for p in range(NP):
                            ps = ps_q.tile([128, 512], f32, tag="psq", name="psq")
                            for kc in range(KC):
                                nc.tensor.matmul(
                                    ps[:],
                                    wq_sb[:, p, kc, :],
                                    xT[:, kc, :],
                                    start=(kc == 0), stop=(kc == KC - 1),
                                )
                            if p % 2 == 0:
                                nc.vector.tensor_copy(qT[:, p, :], ps[:])
                            else:
                                nc.scalar.activation(qT[:, p, :], ps[:], Ident)

                # ---- Phase C: attention against gathered K/V
                wop_cm = tc.tile_pool(name="wop", bufs=1)
                wop = wop_cm.__enter__()
                wosb = wop.tile([128, KC, D], bf16)
                nc.gpsimd.dma_start(wosb[:], wo_d.ap())
                with (
                    tc.tile_pool(name="vsb", bufs=1) as vsbp,
                    tc.tile_pool(name="at", bufs=6) as atpool,
                    tc.tile_pool(name="rec", bufs=3) as recpool,
                    tc.tile_pool(name="ps_s", bufs=2, space="PSUM") as ps_s,
                    tc.tile_pool(name="ps_o", bufs=3, space="PSUM") as ps_o,
                    tc.tile_pool(name="ps_r", bufs=1, space="PSUM") as ps_r,
                ):
                    # prefetch ALL gathered K^T and V on parallel queues;
                    # rank-0 slices first so kt 0..3 unblock immediately
                    ktp_all = vsbp.tile([128, NP, 4, 512], bf16)
                    v_sb = vsbp.tile([128, KTILES, H, 65], bf16)
                    # tiny p=0 K slices first so scores p0 starts at once,
                    # then rank-0 V (first attnV), then the bulk
                    for r in range(4):
                        (nc.sync if r % 2 == 0 else nc.scalar).dma_start(
                            ktp_all[:, 0, r, :], comb_out[r, :, 0:512]
                        )
                    nc.gpsimd.dma_start(
                        v_sb[:, 0:4, :, :],
                        comb_out[0, :, KCOLS:].rearrange(
                            "p (t h v) -> p t h v", t=4, h=H
                        ),
                    )
                    kq = [nc.sync, nc.scalar, nc.gpsimd, nc.sync]
                    for r in range(4):
                        kq[r].dma_start(
                            ktp_all[:, 1:NP, r, :],
                            comb_out[r, :, 512:KCOLS].rearrange(
                                "p (np t) -> p np t", np=NP - 1
                            ),
                        )
                        if r > 0:
                            kq[(r + 1) % 4].dma_start(
                                v_sb[:, 4 * r:4 * (r + 1), :, :],
                                comb_out[r, :, KCOLS:].rearrange(
                                    "p (t h v) -> p t h v", t=4, h=H
                                ),
                            )
                    for p in range(NP):
                        po = [
                            ps_o.tile([65, TOK], f32, tag="po", name=f"po{p}_{hh}")
                            for hh in range(2)
                        ]
                        for g in range(8):
                            for hh in range(2):
                                sT = ps_s.tile([128, 2, 512], f32, tag="sT", name="sT")
                                for j in range(2):
                                    kt = 2 * g + j
                                    nc.tensor.matmul(
                                        sT[:, j, :],
                                        ktp_all[hh * 64:(hh + 1) * 64, p, :, :]
                                        .rearrange("p r t -> p (r t)")[
                                            :, kt * 128:(kt + 1) * 128],
                                        qT[hh * 64:(hh + 1) * 64, p, :],
                                        tile_position=(hh * 64, 0),
                                    )
                                at = atpool.tile([128, 2, 512], bf16, tag="at", name="at")
                                nc.scalar.activation(at[:], sT[:], Exp, scale=0.125)
                                for j in range(2):
                                    kt = 2 * g + j
                                    nc.tensor.matmul(
                                        po[hh][:],
                                        v_sb[:, kt, 2 * p + hh, :],
                                        at[:, j, :],
                                        start=(kt == 0), stop=(kt == KTILES - 1),
                                    )
                        for hh in range(2):
                            rec = recpool.tile([1, TOK], f32r, tag="rec", name="rec")
                            with nc.allow_low_precision(reason="f32r"):
                                nc.vector.reciprocal(rec[:], po[hh][64:65, :])
                            rp = ps_r.tile([64, TOK], f32, tag="rp", name="rp")
                            nc.tensor.matmul(rp[:], ones64[:], rec[:])
                            rsb = recpool.tile([64, TOK], f32, tag="rsb", name="rsb")
                            nc.vector.tensor_copy(rsb[:], rp[:])
                            nc.vector.tensor_mul(
                                o_norm[hh * 64:(hh + 1) * 64, p, :],
                                po[hh][0:64, :],
                                rsb[:],
                            )

                # ---- Phase D+E head: Wo + residual + LN1 + x1T + FFN1
                # (residual adds ride the PSUM accumulation as identity
                # matmuls; LayerNorm stats read PSUM directly)
                with (
                    tc.tile_pool(name="lnd", bufs=1) as lnd,
                    tc.tile_pool(name="ht", bufs=1) as htp,
                ):
                    hT = htp.tile([128, MH, TOK], bf16)
                    psos = []
                    with tc.tile_pool(name="ps_wo", bufs=1, space="PSUM") as ps_wo:
                        for mt in range(4):
                            pso = ps_wo.tile([128, D], f32, tag=f"pso{mt}",
                                             name=f"pso{mt}")
                            for kc in range(KC):
                                for ncc in range(2):
                                    nc.tensor.matmul(
                                        pso[:, ncc * 512:(ncc + 1) * 512],
                                        o_norm[:, kc, mt * 128:(mt + 1) * 128],
                                        wosb[:, kc, ncc * 512:(ncc + 1) * 512],
                                        start=(kc == 0), stop=False,
                                    )
                            for ncc in range(2):
                                nc.tensor.matmul(
                                    pso[:, ncc * 512:(ncc + 1) * 512],
                                    identb[:],
                                    xfb[:, mt, ncc * 512:(ncc + 1) * 512],
                                    start=False, stop=(ncc == 1),
                                )
                            psos.append(pso)
                        ln_quad(lnd, [pso[:] for pso in psos], g1bc, h1bc,
                                [x1[:, mt, :] for mt in range(4)])
                    with tc.tile_pool(name="ps_t2", bufs=2, space="PSUM") as ps_t2:
                        for mt in range(4):
                            for half in range(2):
                                ps = ps_t2.tile([128, 4, 128], f32, tag="trp2",
                                                name="trp2")
                                for q in range(4):
                                    dc = half * 4 + q
                                    nc.tensor.transpose(
                                        ps[:, q, :],
                                        x1[:, mt, dc * 128:(dc + 1) * 128],
                                        ident[:],
                                    )
                                nc.vector.tensor_copy(
                                    x1T[:, half * 4:(half + 1) * 4,
                                        mt * 128:(mt + 1) * 128],
                                    ps[:],
                                )

                    # FFN1, streaming W1 in mh-pairs; also cast x1 -> bf16
                    # for the FFN2 residual matmul
                    with (
                        tc.tile_pool(name="w1p", bufs=4) as w1p,
                        tc.tile_pool(name="ps_f1", bufs=2, space="PSUM") as ps_f1,
                    ):
                        x1b = midp.tile([128, 4, D], bf16)
                        nc.vector.tensor_copy(
                            x1b[:].rearrange("p a d -> p (a d)"),
                            x1[:].rearrange("p a d -> p (a d)"),
                        )
                        w1t = None
                        for mh in range(MH):
                            if mh % 2 == 0:
                                w1t = w1p.tile([128, KC, 256], bf16, tag="w1t",
                                               name="w1t")
                                (nc.sync if mh % 4 == 0 else nc.scalar).dma_start(
                                    w1t[:],
                                    w1_d.ap()[:, :, mh * 128:(mh + 2) * 128],
                                )
                            ps = ps_f1.tile([128, 512], f32, tag="psf1", name="psf1")
                            for dc in range(KC):
                                nc.tensor.matmul(
                                    ps[:],
                                    w1t[:, dc, (mh % 2) * 128:(mh % 2 + 1) * 128],
                                    x1T[:, dc, :],
                                    start=(dc == 0), stop=(dc == KC - 1),
                                )
                            if mh % 2 == 0:
                                nc.scalar.activation(
                                    hT[:, mh, :], ps[:], Relu,
                                    bias=b1c[:, mh:mh + 1],
                                )
                            else:
                                nc.vector.tensor_scalar(
                                    out=hT[:, mh, :], in0=ps[:],
                                    scalar1=b1c[:, mh:mh + 1], scalar2=0.0,
                                    op0=Alu.add, op1=Alu.max,
                                )

                    # ---- Phase E tail: FFN2 (+bias+residual in PSUM),
                    # LayerNorm straight out of PSUM, store
                    with (
                        tc.tile_pool(name="w2p", bufs=4) as w2p,
                        tc.tile_pool(name="ps_f2", bufs=1, space="PSUM") as ps_f2,
                        tc.tile_pool(name="outp", bufs=1) as outp,
                    ):
                        psy = [
                            ps_f2.tile([128, D], f32, tag=f"py{mt}", name=f"py{mt}")
                            for mt in range(4)
                        ]
                        for mh in range(MH):
                            w2t = w2p.tile([128, D], bf16, tag="w2t", name="w2t")
                            (nc.sync if mh % 2 == 0 else nc.scalar).dma_start(
                                w2t[:], w2_d.ap()[:, mh, :]
                            )
                            for mt in range(4):
                                for ncc in range(2):
                                    nc.tensor.matmul(
                                        psy[mt][:, ncc * 512:(ncc + 1) * 512],
                                        hT[:, mh, mt * 128:(mt + 1) * 128],
                                        w2t[:, ncc * 512:(ncc + 1) * 512],
                                        start=(mh == 0), stop=False,
                                    )
                        for mt in range(4):
                            for ncc in range(2):
                                nc.tensor.matmul(
                                    psy[mt][:, ncc * 512:(ncc + 1) * 512],
                                    ones128[:],
                                    b2r[:, ncc * 512:(ncc + 1) * 512],
                                    start=False, stop=False,
                                )
                                nc.tensor.matmul(
                                    psy[mt][:, ncc * 512:(ncc + 1) * 512],
                                    identb[:],
                                    x1b[:, mt, ncc * 512:(ncc + 1) * 512],
                                    start=False, stop=(ncc == 1),
                                )
                        t2s = [
                            outp.tile([128, D], f32, tag=f"t2{mt}", name=f"t2{mt}")
                            for mt in range(4)
                        ]

                        def store_final(mt):
                            (nc.sync if mt % 2 == 0 else nc.scalar).dma_start(
                                y_d.ap()[mt * 128:(mt + 1) * 128, :], t2s[mt][:]
                            )
                        ln_quad(lnd, [p[:] for p in psy], g2bc, h2bc,
                                [t[:] for t in t2s], finals=store_final)
                wop_cm.__exit__(None, None, None)
    nc.compile()
    return nc


def _in_maps(x, Wq, Wk, Wv, Wo, ln1_g, ln1_b, W1, b1, W2, b2, ln2_g, ln2_b):
    import ml_dtypes

    bf16 = ml_dtypes.bfloat16
    x = np.ascontiguousarray(np.asarray(x, np.float32))

    def to_sb(w, ncols):
        # [D_in, N] -> [128, D_in//128, N] partition-major layout, bf16
        w = np.asarray(w, np.float32).reshape(-1, 128, ncols).transpose(1, 0, 2)
        return np.ascontiguousarray(w.astype(bf16))

    def qk_layout(w):
        # [H, D, DK] -> [D, H*DK] -> [128, NP, KC, 128] (partition, head-pair,
        # contraction-block, head-col) so per-p loads are contiguous
        w = np.asarray(w, np.float32).transpose(1, 0, 2).reshape(D, H * DK)
        w = w.reshape(KC, 128, NP, 128).transpose(1, 2, 0, 3)
        return np.ascontiguousarray(w.astype(bf16))

    wv2 = np.asarray(Wv, np.float32).transpose(1, 0, 2).reshape(D, H * DVH)
    wv_r = np.ascontiguousarray(
        wv2.reshape(KC, 128, 2, 512).transpose(1, 2, 0, 3).astype(bf16)
    )
    bcast = lambda v: np.ascontiguousarray(
        np.broadcast_to(np.asarray(v, np.float32), (128, D))
    )
    common = {
        "wq_r": qk_layout(Wq), "wk_r": qk_layout(Wk),
        "wv_r": wv_r,
        "wo_r": to_sb(np.asarray(Wo, np.float32), D),
        "w1_r": to_sb(np.asarray(W1, np.float32), DFF),
        "w2_r": to_sb(np.asarray(W2, np.float32), D),
        "b1c": np.ascontiguousarray(np.asarray(b1, np.float32).reshape(MH, 128).T),
        "b2r": np.ascontiguousarray(np.asarray(b2, np.float32).reshape(1, D)),
        "g1bc": bcast(ln1_g), "h1bc": bcast(ln1_b),
        "g2bc": bcast(ln2_g), "h2bc": bcast(ln2_b),
        "ident": np.eye(128, dtype=np.float32),
        "ones64": np.ones((1, 64), np.float32),
        "ones128": np.ones((1, 128), np.float32),
    }
    in_maps = []
    for c in range(8):
        b, q0 = c // 4, TOK * (c % 4)
        m = dict(common)
        m["xb"] = np.ascontiguousarray(x[b, q0:q0 + TOK, :])
        in_maps.append(m)
    return in_maps


def kernel(x, Wq, Wk, Wv, Wo, ln1_g, ln1_b, W1, b1, W2, b2, ln2_g, ln2_b):
    from concourse.bass_utils import run_bass_kernel_spmd

    if "nc" not in _CACHE:
        _CACHE["nc"] = _build()
    nc = _CACHE["nc"]
    in_maps = _in_maps(x, Wq, Wk, Wv, Wo, ln1_g, ln1_b, W1, b1, W2, b2, ln2_g, ln2_b)
    res = run_bass_kernel_spmd(nc, in_maps, core_ids=list(range(8)))
    out = np.empty((B, S, D), np.float32)
    for c in range(8):
        b, q0 = c // 4, TOK * (c % 4)
        out[b, q0:q0 + TOK, :] = res.results[c]["y_part"]
    return out
